# revision 26
# baseline (speedup 1.0000x reference)
"""Block-sparse linear kernel for Trainium2 (8 NeuronCores, Bass/Tile).

Computes out[n, ob*BS:(ob+1)*BS] += x[n, ib*BS:(ib+1)*BS] @ W[k] for each
nonzero block k with indices (ob, ib), plus bias — data-parallel over the
flattened row dim N across 8 cores (weights replicated, indices baked into
the schedule host-side).

Strategy (host-side schedule specialization from the index tensors):
  - Group input-blocks (ibs) into *families* with identical sets of
    output-blocks (obs); merge overlapping families into superfamilies.
    For the canonical every-10th-block pattern the families are 5 disjoint
    complete-bipartite groups (7,7,6,6,6 ibs x 13,13,13,13,12 obs).
  - Pair ibs within a family: each pair is one K=128 stationary operand
    (two 64-feature x slices, transposed host-side), streaming a
    [128, n_obs*64] stacked-weight moving operand -> full PE utilization.
  - *Unit-major* emission: each unit runs all row-tiles back-to-back, so
    every arriving stream byte unlocks 4x its column count in PE work —
    the PE never outruns the sequential input stream (no cold-phase
    starvation, no HAM re-throttle oscillation).
  - The two leftover odd-family singles (K=64) share one weight block
    (rows 0-63 / 64-127) across adjacent groups, removing their
    zero-fill DMA; at the group boundary the two K=64 matmuls land in
    different PE row-groups and overlap.
  - PSUM accumulates each (group, row-tile) over all units via matmul
    start/stop flags; PSUM->SBUF evictions alternate DVE/ACT; output
    flushes go per (group, row-tile) and alternate between the Sync and
    Scalar HWDGE rings so output DMA drains on a separate queue from the
    input stream.  The final superfamily is split in half and the last
    group runs row-tile-outer so its evict+flush chain pipelines behind
    the remaining matmuls instead of stacking up after the last one.
  - One combined input tensor holds stacked weights and transposed x
    slices in exact consumption order: a single sequential DMA stream on
    the Sync ring delivers data just-in-time, with PE warm-up dummies
    flipping the HAM clock gate to 8/8 during the load head.
  - bf16 inputs and bf16 output staging halve DMA traffic (measured
    rel err ~2.9e-3 vs the f32 reference); set KDTYPE=f32r / KOUT=f32 for
    TF32-like ~1.5e-4 at ~1.4x the time.
"""

import os
import numpy as np
import ml_dtypes
from collections import defaultdict
from contextlib import ExitStack

from concourse import bass_utils, bacc, mybir
import concourse.tile as tile

N_CORES = 8
P = 128            # partitions / row-tile size
CH = 512           # psum-bank matmul chunk (512 f32 cols = 1 bank)
SEG_MAX_OBS = int(os.environ.get("KSEG", "16"))  # blocks per psum segment
F32R = mybir.dt.float32r
F32 = mybir.dt.float32
BF16 = mybir.dt.bfloat16

KDTYPE = os.environ.get("KDTYPE", "bf16")
DT_IN = BF16 if KDTYPE == "bf16" else F32R
NP_IN = ml_dtypes.bfloat16 if KDTYPE == "bf16" else np.float32
KOUT = os.environ.get("KOUT", "bf16")
DT_OUT = BF16 if KOUT == "bf16" else F32
NP_OUT = ml_dtypes.bfloat16 if KOUT == "bf16" else np.float32

_CACHE = {}
LAST_RESULT = None


def _build_schedule(N, F, OUT_F, BS, out_idx, in_idx):
    """Pure-index schedule: families, pairs, shared singles, groups."""
    n_ib = F // BS
    n_ob = OUT_F // BS
    assert F % BS == 0 and OUT_F % BS == 0

    # (ob, ib) -> list of weight slots k (duplicates summed host-side)
    wslots = defaultdict(list)
    for k, (ob, ib) in enumerate(zip(out_idx, in_idx)):
        ob, ib = int(ob), int(ib)
        assert 0 <= ob < n_ob and 0 <= ib < n_ib
        wslots[(ob, ib)].append(k)

    obs_by_ib = defaultdict(set)
    for (ob, ib) in wslots:
        obs_by_ib[ib].add(ob)

    # families: ibs with identical obs sets
    fam_map = defaultdict(list)
    for ib in sorted(obs_by_ib):
        fam_map[frozenset(obs_by_ib[ib])].append(ib)
    families = [(sorted(obs), ibs) for obs, ibs in fam_map.items()]

    # union-find over obs to merge overlapping families into superfamilies
    parent = {}

    def find(a):
        while parent[a] != a:
            parent[a] = parent[parent[a]]
            a = parent[a]
        return a

    for obs, _ in families:
        for ob in obs:
            parent.setdefault(ob, ob)
        r = find(obs[0])
        for ob in obs[1:]:
            parent[find(ob)] = r
    sf_map = defaultdict(lambda: {"obs": set(), "fams": []})
    for obs, ibs in families:
        root = find(obs[0])
        sf_map[root]["obs"].update(obs)
        sf_map[root]["fams"].append((obs, ibs))
    superfams = sorted(sf_map.values(), key=lambda s: min(s["obs"]))

    # xt tile table: pairs (full K=128) and packed singles (K=64 halves)
    xt_tiles = []      # per tile: list of (rowbase, ib) entries
    pair_units = defaultdict(list)  # superfam idx -> [(tile, rowbase, krows, ibs)]
    singles = []                    # (superfam idx, ib)
    for sfi, sf in enumerate(superfams):
        for obs, ibs in sf["fams"]:
            for i in range(0, len(ibs) - 1, 2):
                t = len(xt_tiles)
                xt_tiles.append([(0, ibs[i]), (64, ibs[i + 1])])
                pair_units[sfi].append((t, 0, 128, (ibs[i], ibs[i + 1])))
            if len(ibs) % 2:
                singles.append((sfi, ibs[-1]))
    single_info = {}   # index into singles -> (tile, rowbase)
    for j in range(0, len(singles), 2):
        t = len(xt_tiles)
        entries = [(0, singles[j][1])]
        single_info[j] = (t, 0)
        if j + 1 < len(singles):
            entries.append((64, singles[j + 1][1]))
            single_info[j + 1] = (t, 64)
        xt_tiles.append(entries)

    # share one weight block between the two odd-family singles when they
    # belong to different single-segment superfams: A's single is its
    # group's last unit, B's single is the next group's first unit.
    fuse = None
    if len(singles) == 2:
        sfa, sfb = singles[0][0], singles[1][0]
        if sfa != sfb and len(superfams[sfa]["obs"]) <= SEG_MAX_OBS \
                and len(superfams[sfb]["obs"]) <= SEG_MAX_OBS:
            fuse = (sfa, sfb)
    if fuse is None:
        for j, (sfi, ib) in enumerate(singles):
            t, rb = single_info[j]
            pair_units[sfi].append((t, rb, 64, (ib,)))

    # superfam emission order: the two shared-single superfams adjacent and
    # first; smallest of the rest last (smallest output tail)
    sfis = list(range(len(superfams)))
    if fuse is not None:
        sfis = [fuse[0], fuse[1]] + [i for i in sfis if i not in fuse]
    rest = [i for i in sfis if fuse is None or i not in fuse]
    if rest:
        sm = min(rest, key=lambda i: len(superfams[i]["obs"]))
        sfis.remove(sm)
        sfis.append(sm)

    n_pad = (-N) % (N_CORES * P)
    rows_per_core = (N + n_pad) // N_CORES
    rt_count = rows_per_core // P
    Nc = rows_per_core

    # groups + combined-input layout + out layout.
    # group = one (superfam, obs-chunk): units emitted unit-major, one
    # psum tile per row-tile, evict+flush at group end.
    # unit step = (c0, c1, lhs_rowbase, krows, lhs_col, w_col_lo, start, stop)
    in_blocks = []   # ("w", col, rb, uibs, seg_obs, L) | ("w2", col, specA, specB, Lw)
    xt_off = {}
    in_cols = 0
    out_cols = 0
    groups = []
    cuts = []
    w2_col = None    # set when group A places the shared single block

    def place_x(t):
        nonlocal in_cols
        if t not in xt_off:
            xt_off[t] = in_cols
            in_blocks.append(("x", in_cols, t))
            in_cols += Nc
        return xt_off[t]

    for sfi in sfis:
        sf_obs = sorted(superfams[sfi]["obs"])
        # split the final superfam in half: the last group's eviction+flush
        # chain trails the PE, so a smaller final batch shortens the tail
        seg_max = SEG_MAX_OBS
        if sfi == sfis[-1] and 1 < len(sf_obs) <= SEG_MAX_OBS \
                and (fuse is None or sfi not in fuse):
            seg_max = (len(sf_obs) + 1) // 2
        for s0 in range(0, len(sf_obs), seg_max):
            seg_obs = sf_obs[s0:s0 + seg_max]
            L = len(seg_obs) * BS
            units = []

            def add_unit(t, rb, kr, wc, lc):
                steps = []
                for c0 in range(0, L, CH):
                    c1 = min(c0 + CH, L)
                    steps.append((c0, c1, rb, kr, lc, wc + c0))
                units.append(steps)

            if fuse is not None and sfi == fuse[1]:
                # B's shared single first (block already placed by group A)
                jb = 1 if singles[1][0] == sfi else 0
                t, rb = single_info[jb]
                lc = place_x(t)
                assert w2_col is not None
                add_unit(t, rb, 64, w2_col, lc)
            for (t, rb, kr, uibs) in pair_units[sfi]:
                col = in_cols
                in_blocks.append(("w", col, rb, tuple(uibs), tuple(seg_obs), L))
                in_cols += L
                lc = place_x(t)
                add_unit(t, rb, kr, col, lc)
                if not groups and len(units) == 1 and not cuts:
                    cuts.append(in_cols)
            if fuse is not None and sfi == fuse[0]:
                # A's shared single last; place the shared block here
                ja = 0 if singles[0][0] == sfi else 1
                jb = 1 - ja
                t, rb = single_info[ja]
                tb, rbb = single_info[jb]
                sfb = singles[jb][0]
                obsB = sorted(superfams[sfb]["obs"])
                LB = len(obsB) * BS
                Lw = max(L, LB)
                w2_col = in_cols
                in_blocks.append((
                    "w2", w2_col,
                    (rb, singles[ja][1], tuple(seg_obs), L),
                    (rbb, singles[jb][1], tuple(obsB), LB), Lw))
                in_cols += Lw
                lc = place_x(t)
                add_unit(t, rb, 64, w2_col, lc)
            assert units, f"superfam {sfi} has no units"
            groups.append({
                "units": units, "L": L, "out_base": out_cols,
                "obs": seg_obs,
            })
            out_cols += L
    if not cuts:
        cuts.append(in_cols)

    # chunk the input stream at ~CHUNK_COLS boundaries between the cuts
    CHUNK_COLS = int(os.environ.get("KCHUNK", "3400"))
    cut_set = set(cuts)
    block_edges = sorted({b[1] for b in in_blocks} | {in_cols} | cut_set)
    load_plan = []
    prev = 0
    for edge in block_edges[1:]:
        if edge in cut_set or edge - prev >= CHUNK_COLS or edge == in_cols:
            load_plan.append(("in", prev, edge))
            prev = edge
    assert prev == in_cols

    return {
        "N": N, "F": F, "OUT_F": OUT_F, "BS": BS,
        "wslots": dict(wslots),
        "xt_tiles": xt_tiles,
        "in_blocks": in_blocks, "in_cols": in_cols,
        "groups": groups, "out_cols": out_cols,
        "rows_per_core": rows_per_core, "rt_count": rt_count,
        "load_plan": load_plan,
    }


def _build_nc(meta):
    """Emit the Bass/Tile module for a schedule (value-independent)."""
    Nc = meta["rows_per_core"]
    INC = meta["in_cols"]
    OUTC = meta["out_cols"]
    rt_count = meta["rt_count"]

    nc = bacc.Bacc("TRN2", target_bir_lowering=False, debug=False)
    in_d = nc.dram_tensor("inp", [P, INC], DT_IN, kind="ExternalInput")
    out_d = nc.dram_tensor("out", [Nc, OUTC], DT_OUT, kind="ExternalOutput")

    # warm-up matmuls (~0.63us each at the cold half-clock) keep the PE
    # active until the first input chunk's DMA semaphore fires (jitters
    # ~10.5-12.5us incl. receipt); an idle gap there resets the HAM clock
    # ramp and costs far more than the warmups do.
    n_warm = int(os.environ.get("KWARM", "8"))

    with tile.TileContext(nc) as tc, ExitStack() as ctx:
        # one PSUM tile per 512-col matmul chunk: every tile is a single
        # bank, so the pool holds 8 buffers -> twice the pipeline depth of
        # 2-bank whole-segment tiles (next group's matmuls don't wait on
        # the previous group's evictions)
        ps_cols = min(CH, max(g["L"] for g in meta["groups"]))
        ps_bufs = 8
        in_pool = ctx.enter_context(tc.tile_pool(name="in", bufs=1))
        warm_pool = ctx.enter_context(tc.tile_pool(name="wm", bufs=1))
        psum_pool = ctx.enter_context(
            tc.tile_pool(name="ps", bufs=ps_bufs, space="PSUM"))
        out_pool = ctx.enter_context(tc.tile_pool(name="ot", bufs=1))

        inp = in_pool.tile([P, INC], DT_IN)

        # PE warm-up: dummy matmuls on a memset scratch tile (no DMA deps)
        # run during the input-load head and flip HAM to 8/8 early.
        if n_warm:
            wsb = warm_pool.tile([P, 512], DT_IN)
            nc.gpsimd.memset(wsb[:].bitcast(F32), 0)
            wps = psum_pool.tile([P, ps_cols], F32, tag="mm")
            # coarse 512-col warmups, then fine 128-col ones so the PE
            # becomes free within ~0.15us of the first chunk's semaphore;
            # the fine tail also insures against late semaphores (receipt
            # jitter) that would otherwise idle the PE and reset the HAM
            # clock ramp.
            warm_cols = [512] * (n_warm - 2) + [128] * 14
            for wn in warm_cols:
                wn = min(wn, ps_cols)
                nc.tensor.matmul(wps[:, :wn], wsb[:, :P], wsb[:, :wn],
                                 start=True, stop=True)

        # sequential input stream in consumption order (Sync HWDGE ring)
        for (_, a, b) in meta["load_plan"]:
            nc.sync.dma_start(out=inp[:, a:b], in_=in_d[:, a:b])

        # unit-major: each unit streams all row tiles back-to-back so the
        # PE tracks the input stream with 4x work per streamed column.
        out_sbs = [out_pool.tile([P, OUTC], DT_OUT, name=f"osb{r}", tag=f"osb{r}")
                   for r in range(rt_count)]
        ev_i = 0
        flush_i = 0
        n_groups = len(meta["groups"])
        for gi, g in enumerate(meta["groups"]):
            L = g["L"]
            ob = g["out_base"]
            last_group = gi == n_groups - 1
            n_units = len(g["units"])
            cks = [(c0, c1) for (c0, c1, _rb, _kr, _lc, _w) in g["units"][0]]
            if last_group:
                # rt-outer for the final group: its data is long-resident
                # in SBUF, and per-rt evict+flush pipelines behind the next
                # rt's matmuls instead of all stacking up after the last MM
                for rt in range(rt_count):
                    ptiles = {}
                    for ui, steps in enumerate(g["units"]):
                        for (c0, c1, rb, kr, lc, wlo) in steps:
                            if c0 not in ptiles:
                                ptiles[c0] = psum_pool.tile(
                                    [P, c1 - c0], F32, name="psmm", tag="mm")
                            nc.tensor.matmul(
                                ptiles[c0][:, 0:c1 - c0],
                                inp[rb:rb + kr, lc + rt * P: lc + (rt + 1) * P],
                                inp[rb:rb + kr, wlo:wlo + (c1 - c0)],
                                start=ui == 0, stop=ui == n_units - 1,
                                skip_group_check=True)
                    # fixed engine assignment: the final rt (3) evicts on
                    # DVE and flushes on the Sync ring — both idle by then —
                    # so the closing chain never queues behind earlier
                    # flushes on the Scalar engine/ring
                    for (c0, c1) in cks:
                        dst = out_sbs[rt][:, ob + c0: ob + c1]
                        src = ptiles[c0][:, 0:c1 - c0]
                        if rt % 2 == 1:
                            nc.vector.tensor_copy(out=dst, in_=src)
                        else:
                            nc.scalar.copy(dst, src)
                        ev_i += 1
                    eng = nc.sync if rt % 2 == 1 else nc.scalar
                    eng.dma_start(
                        out=out_d[rt * P:(rt + 1) * P, ob:ob + L],
                        in_=out_sbs[rt][:, ob:ob + L])
                    flush_i += 1
                continue
            psums = {}
            for ui, steps in enumerate(g["units"]):
                st = ui == 0
                sp = ui == n_units - 1
                for rt in range(rt_count):
                    for (c0, c1, rb, kr, lc, wlo) in steps:
                        if (rt, c0) not in psums:
                            psums[(rt, c0)] = psum_pool.tile(
                                [P, c1 - c0], F32, name="psmm", tag="mm")
                        nc.tensor.matmul(
                            psums[(rt, c0)][:, 0:c1 - c0],
                            inp[rb:rb + kr, lc + rt * P: lc + (rt + 1) * P],
                            inp[rb:rb + kr, wlo:wlo + (c1 - c0)],
                            start=st, stop=sp, skip_group_check=True)
            for rt in range(rt_count):
                for (c0, c1) in cks:
                    dst = out_sbs[rt][:, ob + c0: ob + c1]
                    src = psums[(rt, c0)][:, 0:c1 - c0]
                    if ev_i % 2 == 0:
                        nc.vector.tensor_copy(out=dst, in_=src)
                    else:
                        nc.scalar.copy(dst, src)
                    ev_i += 1
                if gi == n_groups - 2:
                    # second-to-last group flushes on the Sync ring only,
                    # keeping the Scalar engine/ring clear ahead of the
                    # last group's tail evict+flush chain
                    eng = nc.sync
                else:
                    eng = nc.sync if flush_i % 2 == 0 else nc.scalar
                eng.dma_start(
                    out=out_d[rt * P:(rt + 1) * P, ob:ob + L],
                    in_=out_sbs[rt][:, ob:ob + L])
                flush_i += 1
    nc.compile()
    return nc


def _host_tensors(meta, x2, weight):
    """Build per-core combined input arrays (values only)."""
    BS = meta["BS"]
    Nc = meta["rows_per_core"]
    Ntot = Nc * N_CORES

    if x2.shape[0] < Ntot:
        x2 = np.concatenate(
            [x2, np.zeros((Ntot - x2.shape[0], x2.shape[1]), np.float32)], axis=0)

    wsum = {}
    for (ob_ib, ks) in meta["wslots"].items():
        w = weight[ks[0]]
        for k in ks[1:]:
            w = w + weight[k]
        wsum[ob_ib] = np.ascontiguousarray(w, dtype=np.float32)

    # weight part is identical across cores: fill once
    base = np.zeros((P, meta["in_cols"]), np.float32)
    for blk in meta["in_blocks"]:
        if blk[0] == "w":
            _, col, rb, uibs, seg_obs, _L = blk
            for r, ib in enumerate(uibs):
                row0 = rb + r * 64
                for j, ob in enumerate(seg_obs):
                    w = wsum.get((ob, ib))
                    if w is not None:
                        base[row0:row0 + 64, col + j * BS: col + (j + 1) * BS] = w
        elif blk[0] == "w2":
            _, col, specA, specB, _Lw = blk
            for (rb, ib, obs, _L) in (specA, specB):
                for j, ob in enumerate(obs):
                    w = wsum.get((ob, ib))
                    if w is not None:
                        base[rb:rb + 64, col + j * BS: col + (j + 1) * BS] = w

    in_all = []
    for c in range(N_CORES):
        xs = x2[c * Nc:(c + 1) * Nc]           # [Nc, F]
        comb = base.copy()
        for blk in meta["in_blocks"]:
            if blk[0] != "x":
                continue
            col, t = blk[1], blk[2]
            for (rbase, ib) in meta["xt_tiles"][t]:
                comb[rbase:rbase + 64, col:col + Nc] = \
                    xs[:, ib * BS:(ib + 1) * BS].T
        in_all.append(np.ascontiguousarray(comb.astype(NP_IN)))
    return in_all


def kernel(**inputs):
    global LAST_RESULT
    x = np.asarray(inputs["x"], dtype=np.float32)
    weight = np.asarray(inputs["weight"], dtype=np.float32)
    bias = np.asarray(inputs["bias"], dtype=np.float32)
    out_idx = np.asarray(inputs["out_block_idx"]).astype(np.int64)
    in_idx = np.asarray(inputs["in_block_idx"]).astype(np.int64)

    B, S, F = x.shape
    N = B * S
    BS = weight.shape[1]
    OUT_F = bias.shape[0]
    x2 = np.ascontiguousarray(x.reshape(N, F))

    key = (N, F, OUT_F, BS, out_idx.tobytes(), in_idx.tobytes())
    if key not in _CACHE:
        meta = _build_schedule(N, F, OUT_F, BS, out_idx, in_idx)
        nc = _build_nc(meta)
        _CACHE[key] = (nc, meta)
    nc, meta = _CACHE[key]

    in_all = _host_tensors(meta, x2, weight)
    in_maps = [{"inp": in_all[c]} for c in range(N_CORES)]
    try:
        res = bass_utils.run_bass_kernel_spmd(
            nc, in_maps, core_ids=list(range(N_CORES)))
    except Exception:
        # transient accelerator hiccups (e.g. a wedged core from a prior
        # process) usually clear on retry
        res = bass_utils.run_bass_kernel_spmd(
            nc, in_maps, core_ids=list(range(N_CORES)))
    LAST_RESULT = res

    dev = np.concatenate(
        [np.asarray(res.results[c]["out"]).astype(np.float32)
         for c in range(N_CORES)], axis=0)
    dev = dev[:N]  # drop row padding

    out = np.zeros((N, OUT_F), np.float32)
    for g in meta["groups"]:
        b = g["out_base"]
        for j, ob in enumerate(g["obs"]):
            out[:, ob * BS:(ob + 1) * BS] = dev[:, b + j * BS: b + (j + 1) * BS]
    if bias.any():
        out += bias
    return out.reshape(B, S, OUT_F)
